# revision 30
# baseline (speedup 1.0000x reference)
# Trainium2 Bass kernel for AudioContextAggregation (windowed cross-attention).
#
# Strategy: data-parallel over batch B=16 across 8 NeuronCores (2 batches/core).
# Host folds LayerNorm gamma/beta and the 1/sqrt(hd) attention scale into the
# projection weights/biases (shipped as bf16), zero-pads audio along the frame
# axis so the per-batch window gather becomes a single dynamically-offset DMA,
# and pre-transposes all weights to the K-major layout the PE array wants.
# The K-projection bias is dropped exactly: it shifts every key score of a
# query by the same constant, which softmax cancels.
#
# Device pipeline per batch:
#   kv gather -> LN -> kvT -> K/V projections token-major (N=512) -> kT
#   per 512-token chunk of the 1024 queries:
#     LN(hidden) -> xhatT (DMA transpose) -> Q proj (bf16, N=512) ->
#     scores^T = kT.T @ qT (N=512) -> exp -> row-sums via ones-matmul ->
#     reciprocal -> broadcast-DMA -> ctx^T = (v.T @ expT) * recip ->
#     out proj (+bias via K=1 ones matmul) -> +hidden residual in fp32 -> out
# All matmuls run bf16 (1 cycle/row); LN stats, softmax sums and the residual
# stay fp32.

import numpy as np
import ml_dtypes

import concourse.bass as bass
import concourse.mybir as mybir
import concourse.tile as tile
from concourse import bacc
from concourse.bass import ds
from concourse.bass_utils import run_bass_kernel_spmd
from concourse.masks import make_identity

F32 = mybir.dt.float32
BF16 = mybir.dt.bfloat16
I32 = mybir.dt.int32
AF = mybir.ActivationFunctionType
ALU = mybir.AluOpType
AX = mybir.AxisListType

NCORES = 8
B, T, L, D = 16, 200, 5, 1024
LQ = 1024
H, HD = 8, 128
NB = B // NCORES            # batches per core
WIN_LO, WIN_HI = -4, 6
WIN = WIN_HI - WIN_LO       # 10 frames
KV = WIN * L                # 50 kv tokens
PAD_LO = -WIN_LO            # 4 zero frames in front
PADF = PAD_LO + T + WIN_HI  # 210 padded frames
PADR = PADF * L             # 1050 padded rows
KC = D // 128               # 8 feature chunks of 128
TCH = 512                   # token chunk (matmul N)
NCH = LQ // TCH             # chunks per batch
TPC = TCH // 128            # 128-token tiles per chunk
EPS = 1e-5


def _bcast_part(ap: bass.AP, p: int) -> bass.AP:
    """View a [1, N] AP as [p, N] with partition stride 0 (DMA broadcast)."""
    return bass.AP(tensor=ap.tensor, offset=ap.offset, ap=[[0, p]] + ap.ap[1:])


def _layernorm_stats(nc, pools, x_ap, p):
    """Emit LN stats for x_ap [p, D] (f32). Returns (rstd, neg_mu_rstd) [p,1]."""
    st = pools["stats"]
    nsub = D // 512
    stats = st.tile([128, nsub, 6], F32, tag="bnst")
    xg = x_ap.rearrange("p (s d) -> p s d", s=nsub)
    for s in range(nsub):
        nc.vector.bn_stats(out=stats[:p, s, :], in_=xg[:, s, :])
    mv = st.tile([128, 2], F32, tag="bnmv")
    nc.vector.bn_aggr(out=mv[:p], in_=stats[:p])
    rstd = st.tile([128, 1], F32, tag="rstd")
    # rstd = 1/sqrt(var + eps)
    nc.scalar.activation(out=rstd[:p], in_=mv[:p, 1:2], func=AF.Sqrt,
                         bias=pools["eps"][:p], scale=1.0)
    nc.vector.reciprocal(out=rstd[:p], in_=rstd[:p])
    nmr = st.tile([128, 1], F32, tag="nmr")
    # nmr = -(mean * rstd)
    nc.vector.tensor_scalar(out=nmr[:p], in0=mv[:p, 0:1], scalar1=rstd[:p],
                            scalar2=-1.0, op0=ALU.mult, op1=ALU.mult)
    return rstd, nmr


def build_program() -> bass.Bass:
    nc = bacc.Bacc("TRN2", target_bir_lowering=False, debug=False)

    hid = nc.declare_dram_parameter("hidden", [NB, LQ, D], F32, isOutput=False)
    aud = nc.declare_dram_parameter("audio_pad", [NB, PADR, D], F32, isOutput=False)
    srow = nc.declare_dram_parameter("start_row", [1, NB], I32, isOutput=False)
    wq = nc.declare_dram_parameter("wq", [128, KC, D], BF16, isOutput=False)
    wk = nc.declare_dram_parameter("wk", [128, KC, D], BF16, isOutput=False)
    wv = nc.declare_dram_parameter("wv", [128, KC, D], BF16, isOutput=False)
    wo = nc.declare_dram_parameter("wo", [128, KC, D], BF16, isOutput=False)
    bq = nc.declare_dram_parameter("bq", [128, KC], F32, isOutput=False)
    bo = nc.declare_dram_parameter("bo", [1, D], BF16, isOutput=False)
    out = nc.declare_dram_parameter("out", [NB, LQ, D], F32, isOutput=True)

    with tile.TileContext(nc) as tc:
        _emit(tc, hid, aud, srow, wq, wk, wv, wo, bq, bo, out)
    nc.compile()
    return nc


def _emit(tc, hid, aud, srow, wq, wk, wv, wo, bq, bo, out):
    nc = tc.nc

    with (
        tc.tile_pool(name="const", bufs=1) as const,
        tc.tile_pool(name="wpool", bufs=1) as wpool,
        tc.tile_pool(name="xp", bufs=3) as xp,
        tc.tile_pool(name="xrp", bufs=6) as xrp,
        tc.tile_pool(name="xhp", bufs=8) as xhp,
        tc.tile_pool(name="stats", bufs=8) as stats,
        tc.tile_pool(name="xtp", bufs=2) as xtp,
        tc.tile_pool(name="qtp", bufs=2) as qtp,
        tc.tile_pool(name="etp", bufs=4) as etp,
        tc.tile_pool(name="rrp", bufs=4) as rrp,
        tc.tile_pool(name="rbp", bufs=3) as rbp,
        tc.tile_pool(name="ctp", bufs=2) as ctp,
        tc.tile_pool(name="osp", bufs=2) as osp,
        tc.tile_pool(name="kvp", bufs=1) as kvp,
        tc.tile_pool(name="psum", bufs=8, space="PSUM") as psum,
    ):
        pools = {"stats": stats}

        # ---- constants / weights (one-time) ----
        ident = const.tile([128, 128], BF16, tag="ident")
        make_identity(nc, ident)
        eps_t = const.tile([128, 1], F32, tag="eps")
        nc.vector.memset(eps_t, EPS)
        pools["eps"] = eps_t
        ones_r = const.tile([1, 128], BF16, tag="ones")
        nc.vector.memset(ones_r, 1.0)
        ones_c = const.tile([128, 1], BF16, tag="onesc")
        nc.vector.memset(ones_c, 1.0)

        w_sb = {}
        for name, prm in (("wq", wq), ("wk", wk), ("wv", wv), ("wo", wo)):
            t = wpool.tile([128, KC, D], BF16, tag=name)
            nc.scalar.dma_start(out=t, in_=prm[:])
            w_sb[name] = t
        bq_sb = const.tile([128, KC], F32, tag="bq")
        nc.gpsimd.dma_start(out=bq_sb, in_=bq[:])
        bo_sb = const.tile([1, D], BF16, tag="bo")
        nc.gpsimd.dma_start(out=bo_sb, in_=bo[:])

        srow_sb = const.tile([1, NB], I32, tag="srow")
        nc.gpsimd.dma_start(out=srow_sb, in_=srow[:])
        rows = [
            nc.values_load(srow_sb[0:1, b:b + 1], min_val=0,
                           max_val=(PADF - WIN) * L,
                           skip_runtime_bounds_check=True)
            for b in range(NB)
        ]

        for b in range(NB):
            # ================= KV path (50 tokens) =================
            kv_raw = kvp.tile([KV, D], F32, tag="kv_raw")
            nc.gpsimd.dma_start(out=kv_raw, in_=aud[b, ds(rows[b], KV), :])

            rstd, nmr = _layernorm_stats(nc, pools, kv_raw, KV)
            kvh = kvp.tile([KV, D], BF16, tag="kvh")
            nc.scalar.activation(out=kvh, in_=kv_raw, func=AF.Identity,
                                 bias=nmr[:KV], scale=rstd[:KV])

            # kvT[kc] : [128, KV]  (feature-major normalized kv)
            kvT = kvp.tile([128, KC, KV], BF16, tag="kvT")
            for kc in range(KC):
                pt = psum.tile([128, 512], BF16, tag="ps")
                nc.tensor.transpose(pt[:, :KV], kvh[:, kc * 128:(kc + 1) * 128],
                                    ident[:KV, :KV])
                nc.vector.tensor_copy(out=kvT[:, kc, :], in_=pt[:, :KV])

            # k, v token-major [KV, D] (k bias dropped: softmax-invariant)
            k_tok = kvp.tile([KV, D], BF16, tag="k_tok")
            v_sb = kvp.tile([128, D], BF16, tag="v")
            for dst, wname in ((k_tok, "wk"), (v_sb, "wv")):
                for vc in range(D // TCH):
                    pv = psum.tile([128, 512], F32, tag="ps")
                    for kc in range(KC):
                        nc.tensor.matmul(pv[:KV, :], kvT[:, kc, :],
                                         w_sb[wname][:, kc, vc * TCH:(vc + 1) * TCH],
                                         start=(kc == 0), stop=(kc == KC - 1))
                    nc.vector.tensor_copy(out=dst[:KV, vc * TCH:(vc + 1) * TCH],
                                          in_=pv[:KV, :])
            # duplicate v at partitions 64.. so paired heads can contract there
            nc.sync.dma_start(out=v_sb[64:64 + KV, :], in_=v_sb[:KV, :])

            # kT[h] : [128, 64] per head (feature-major keys, padded 50->64 so
            # head pairs pack one PSUM bank with no junk rows)
            kT = kvp.tile([128, H, 64], BF16, tag="kT")
            nc.gpsimd.memset(kT, 0.0)
            for h in range(H):
                pt = psum.tile([128, 512], BF16, tag="ps")
                nc.tensor.transpose(pt[:, :KV], k_tok[:, h * 128:(h + 1) * 128],
                                    ident[:KV, :KV])
                nc.scalar.activation(out=kT[:, h, :KV], in_=pt[:, :KV],
                                     func=AF.Identity, bias=0.0, scale=1.0)

            # ====== LN for the whole batch up front (overlaps kv stage) ======
            xh_tiles = []
            xr_tiles = []
            for tt8 in range(LQ // 128):
                t0 = tt8 * 128
                # separate residual copy so the LN tile frees early
                x_r = xrp.tile([128, D], F32, tag="xr")
                xr_tiles.append(x_r)
                nc.sync.dma_start(out=x_r, in_=hid[b, t0:t0 + 128, :])

                rstd, nmr = _layernorm_stats(nc, pools, x_r, 128)
                xh = xhp.tile([128, D], BF16, tag="xh")
                nc.scalar.activation(out=xh, in_=x_r, func=AF.Identity,
                                     bias=nmr, scale=rstd)
                xh_tiles.append(xh)

            # ================= queries, 512-token chunks =================
            for ch in range(NCH):
                xt_c = xtp.tile([128, KC, TCH], BF16, tag="xt")
                x_tiles = xr_tiles[ch * TPC:(ch + 1) * TPC]
                for tt in range(TPC):
                    xh = xh_tiles[ch * TPC + tt]
                    # transpose 8 x [128,128] on PE, packed 4 per PSUM bank
                    for g in range(2):
                        pg = psum.tile([128, 512], BF16, tag="ps")
                        for j in range(4):
                            kc = g * 4 + j
                            nc.tensor.transpose(
                                pg[:, j * 128:(j + 1) * 128],
                                xh[:, kc * 128:(kc + 1) * 128], ident)
                        nc.scalar.activation(
                            out=xt_c[:, g * 4:(g + 1) * 4,
                                     tt * 128:(tt + 1) * 128],
                            in_=pg.rearrange("p (j f) -> p j f", j=4),
                            func=AF.Copy, bias=0.0, scale=1.0)

                # Q projection: qT_c[oc] = [128, TCH]
                qT_c = qtp.tile([128, KC, TCH], BF16, tag="qt")
                for oc in range(KC):
                    pq = psum.tile([128, TCH], F32, tag="ps")
                    for kc in range(KC):
                        nc.tensor.matmul(pq,
                                         w_sb["wq"][:, kc, oc * 128:(oc + 1) * 128],
                                         xt_c[:, kc, :],
                                         start=(kc == 0), stop=(kc == KC - 1))
                    nc.scalar.activation(out=qT_c[:, oc, :], in_=pq,
                                         func=AF.Identity,
                                         bias=bq_sb[:, oc:oc + 1], scale=1.0)

                # attention, head pairs: scores^T -> exp -> ctx^T (normalized).
                # Heads 2p/2p+1 land at PSUM partitions 0-63 / 64-127 (kT key
                # dim zero-padded to 64), so exp covers both in one full-lane
                # activation.
                ctxT = ctp.tile([128, KC, TCH], BF16, tag="ct")
                for p in range(H // 2):
                    h0, h1 = 2 * p, 2 * p + 1
                    sc = psum.tile([128, TCH], F32, tag="ps")
                    nc.tensor.matmul(sc[0:64, :], kT[:, h0, :], qT_c[:, h0, :],
                                     start=True, stop=True)
                    nc.tensor.matmul(sc[64:128, :], kT[:, h1, :],
                                     qT_c[:, h1, :], start=True, stop=True,
                                     tile_position=(0, 64))
                    expP = etp.tile([128, TCH], BF16, tag="et")
                    nc.scalar.activation(out=expP, in_=sc, func=AF.Exp,
                                         bias=0.0, scale=1.0)
                    for j, h in ((0, h0), (1, h1)):
                        sm = psum.tile([1, TCH], F32, tag="ps")
                        nc.tensor.matmul(
                            sm, ones_c[64 * j:64 * j + KV, :],
                            expP[64 * j:64 * j + KV, :], start=True, stop=True,
                            tile_position=(64 * j, 0))
                        rrow = rrp.tile([1, TCH], F32, tag="rr")
                        nc.vector.reciprocal_approx_fast(out=rrow, in_=sm)
                        rb = rbp.tile([128, TCH], F32, tag="rb")
                        nc.gpsimd.partition_broadcast(rb, rrow)
                        pc = psum.tile([128, TCH], F32, tag="ps")
                        nc.tensor.matmul(pc,
                                         v_sb[64 * j:64 * j + KV,
                                              h * 128:(h + 1) * 128],
                                         expP[64 * j:64 * j + KV, :],
                                         start=True, stop=True)
                        nc.vector.tensor_mul(out=ctxT[:, h, :], in0=pc, in1=rb)

                # out projection + bias + residual, per 128-token tile
                for tt in range(TPC):
                    t0 = ch * TCH + tt * 128
                    osb = osp.tile([128, D], F32, tag="os")
                    for vc in range(D // TCH):
                        po = psum.tile([128, TCH], F32, tag="ps")
                        for kc in range(KC):
                            nc.tensor.matmul(
                                po, ctxT[:, kc, tt * 128:(tt + 1) * 128],
                                w_sb["wo"][:, kc, vc * TCH:(vc + 1) * TCH],
                                start=(kc == 0), stop=False)
                        nc.tensor.matmul(po, ones_r,
                                         bo_sb[:, vc * TCH:(vc + 1) * TCH],
                                         start=False, stop=True)
                        nc.vector.tensor_add(
                            out=osb[:, vc * TCH:(vc + 1) * TCH], in0=po,
                            in1=x_tiles[tt][:, vc * TCH:(vc + 1) * TCH])
                    nc.sync.dma_start(out=out[b, t0:t0 + 128, :], in_=osb)


# ---------------------------------------------------------------------------
# host side
# ---------------------------------------------------------------------------

_PROG = None


def get_program() -> bass.Bass:
    global _PROG
    if _PROG is None:
        _PROG = build_program()
    return _PROG


def prep_inputs(hidden, audio_features, frame_idx, q_gamma, q_beta, kv_gamma,
                kv_beta, in_proj_w, in_proj_b, out_w, out_b):
    """Host-side sharding + parameter folding. Returns list of per-core maps."""
    hidden = np.asarray(hidden, np.float32)
    audio = np.asarray(audio_features, np.float32)
    fidx = np.asarray(frame_idx).astype(np.int64)
    q_gamma = np.asarray(q_gamma, np.float64)
    q_beta = np.asarray(q_beta, np.float64)
    kv_gamma = np.asarray(kv_gamma, np.float64)
    kv_beta = np.asarray(kv_beta, np.float64)
    w_in = np.asarray(in_proj_w, np.float64)
    b_in = np.asarray(in_proj_b, np.float64)
    w_out = np.asarray(out_w, np.float64)
    b_out = np.asarray(out_b, np.float64)

    Wq, Wk, Wv = w_in[:D], w_in[D:2 * D], w_in[2 * D:]
    bqv, bvv = b_in[:D], b_in[2 * D:]
    s = 1.0 / np.sqrt(HD)

    Wq_f = Wq * q_gamma[None, :] * s
    bq_f = (bqv + Wq @ q_beta) * s
    Wk_f = Wk * kv_gamma[None, :]
    Wv_f = Wv * kv_gamma[None, :]
    bv_f = bvv + Wv @ kv_beta
    bo_f = b_out + w_out @ bv_f

    def chunkT(w):  # [o,d] -> wT [d,o] -> [128, KC, D] (p, c, o)
        wt = np.ascontiguousarray(w.T).astype(np.float32)
        return np.ascontiguousarray(
            wt.reshape(KC, 128, D).transpose(1, 0, 2)).astype(ml_dtypes.bfloat16)

    wq_ship = chunkT(Wq_f)
    wk_ship = chunkT(Wk_f)
    wv_ship = chunkT(Wv_f)
    wo_ship = chunkT(w_out)

    bq_ship = np.ascontiguousarray(
        bq_f.astype(np.float32).reshape(KC, 128).T)
    bo_ship = bo_f.astype(np.float32).astype(ml_dtypes.bfloat16).reshape(1, D)

    # zero-padded audio: frame f -> rows (f+PAD_LO)*L ...; window start row =
    # (idx + WIN_LO + PAD_LO) * L = idx * L
    audio_pad = np.zeros((B, PADR, D), np.float32)
    audio_pad[:, PAD_LO * L:(PAD_LO + T) * L, :] = audio.reshape(B, T * L, D)
    start_row = (fidx * L).astype(np.int32)

    in_maps = []
    for c in range(NCORES):
        b0, b1 = c * NB, (c + 1) * NB
        in_maps.append({
            "hidden": hidden[b0:b1],
            "audio_pad": audio_pad[b0:b1],
            "start_row": start_row[b0:b1].reshape(1, NB),
            "wq": wq_ship, "wk": wk_ship, "wv": wv_ship, "wo": wo_ship,
            "bq": bq_ship, "bo": bo_ship,
        })
    return in_maps


def run(in_maps, **kwargs):
    nc = get_program()
    return run_bass_kernel_spmd(nc, in_maps, list(range(NCORES)), **kwargs)


def kernel(**inputs) -> np.ndarray:
    in_maps = prep_inputs(**inputs)
    res = run(in_maps)
    outs = [res.results[c]["out"] for c in range(NCORES)]
    return np.concatenate(outs, axis=0).astype(np.float32)


# revision 31
# speedup vs baseline: 1.0283x; 1.0283x over previous
# Trainium2 Bass kernel for AudioContextAggregation (windowed cross-attention).
#
# Strategy: data-parallel over batch B=16 across 8 NeuronCores (2 batches/core).
# Host folds LayerNorm gamma/beta and the 1/sqrt(hd) attention scale into the
# projection weights/biases (shipped as bf16), zero-pads audio along the frame
# axis so the per-batch window gather becomes a single dynamically-offset DMA,
# and pre-transposes all weights to the K-major layout the PE array wants.
# The K-projection bias is dropped exactly: it shifts every key score of a
# query by the same constant, which softmax cancels.
#
# Device pipeline per batch:
#   kv gather -> LN -> kvT -> K/V projections token-major (N=512) -> kT
#   per 512-token chunk of the 1024 queries:
#     LN(hidden) -> xhatT (DMA transpose) -> Q proj (bf16, N=512) ->
#     scores^T = kT.T @ qT (N=512) -> exp -> row-sums via ones-matmul ->
#     reciprocal -> broadcast-DMA -> ctx^T = (v.T @ expT) * recip ->
#     out proj (+bias via K=1 ones matmul) -> +hidden residual in fp32 -> out
# All matmuls run bf16 (1 cycle/row); LN stats, softmax sums and the residual
# stay fp32.

import numpy as np
import ml_dtypes

import concourse.bass as bass
import concourse.mybir as mybir
import concourse.tile as tile
from concourse import bacc
from concourse.bass import ds
from concourse.bass_utils import run_bass_kernel_spmd
from concourse.masks import make_identity

F32 = mybir.dt.float32
BF16 = mybir.dt.bfloat16
I32 = mybir.dt.int32
AF = mybir.ActivationFunctionType
ALU = mybir.AluOpType
AX = mybir.AxisListType

NCORES = 8
B, T, L, D = 16, 200, 5, 1024
LQ = 1024
H, HD = 8, 128
NB = B // NCORES            # batches per core
WIN_LO, WIN_HI = -4, 6
WIN = WIN_HI - WIN_LO       # 10 frames
KV = WIN * L                # 50 kv tokens
PAD_LO = -WIN_LO            # 4 zero frames in front
PADF = PAD_LO + T + WIN_HI  # 210 padded frames
PADR = PADF * L             # 1050 padded rows
KC = D // 128               # 8 feature chunks of 128
TCH = 512                   # token chunk (matmul N)
NCH = LQ // TCH             # chunks per batch
TPC = TCH // 128            # 128-token tiles per chunk
EPS = 1e-5


def _bcast_part(ap: bass.AP, p: int) -> bass.AP:
    """View a [1, N] AP as [p, N] with partition stride 0 (DMA broadcast)."""
    return bass.AP(tensor=ap.tensor, offset=ap.offset, ap=[[0, p]] + ap.ap[1:])


def _layernorm_stats(nc, pools, x_ap, p):
    """Emit LN stats for x_ap [p, D] (f32). Returns (rstd, neg_mu_rstd) [p,1]."""
    st = pools["stats"]
    nsub = D // 512
    stats = st.tile([128, nsub, 6], F32, tag="bnst")
    xg = x_ap.rearrange("p (s d) -> p s d", s=nsub)
    for s in range(nsub):
        nc.vector.bn_stats(out=stats[:p, s, :], in_=xg[:, s, :])
    mv = st.tile([128, 2], F32, tag="bnmv")
    nc.vector.bn_aggr(out=mv[:p], in_=stats[:p])
    rstd = st.tile([128, 1], F32, tag="rstd")
    # rstd = 1/sqrt(var + eps)
    nc.scalar.activation(out=rstd[:p], in_=mv[:p, 1:2], func=AF.Sqrt,
                         bias=pools["eps"][:p], scale=1.0)
    nc.vector.reciprocal(out=rstd[:p], in_=rstd[:p])
    nmr = st.tile([128, 1], F32, tag="nmr")
    # nmr = -(mean * rstd)
    nc.vector.tensor_scalar(out=nmr[:p], in0=mv[:p, 0:1], scalar1=rstd[:p],
                            scalar2=-1.0, op0=ALU.mult, op1=ALU.mult)
    return rstd, nmr


def build_program() -> bass.Bass:
    nc = bacc.Bacc("TRN2", target_bir_lowering=False, debug=False)

    hid = nc.declare_dram_parameter("hidden", [NB, LQ, D], F32, isOutput=False)
    aud = nc.declare_dram_parameter("audio_pad", [NB, PADR, D], F32, isOutput=False)
    srow = nc.declare_dram_parameter("start_row", [1, NB], I32, isOutput=False)
    wq = nc.declare_dram_parameter("wq", [128, KC, D], BF16, isOutput=False)
    wk = nc.declare_dram_parameter("wk", [128, KC, D], BF16, isOutput=False)
    wv = nc.declare_dram_parameter("wv", [128, KC, D], BF16, isOutput=False)
    wo = nc.declare_dram_parameter("wo", [128, KC, D], BF16, isOutput=False)
    bq = nc.declare_dram_parameter("bq", [128, KC], F32, isOutput=False)
    bo = nc.declare_dram_parameter("bo", [1, D], BF16, isOutput=False)
    out = nc.declare_dram_parameter("out", [NB, LQ, D], F32, isOutput=True)

    with tile.TileContext(nc) as tc:
        _emit(tc, hid, aud, srow, wq, wk, wv, wo, bq, bo, out)
    nc.compile()
    return nc


def _emit(tc, hid, aud, srow, wq, wk, wv, wo, bq, bo, out):
    nc = tc.nc

    with (
        tc.tile_pool(name="const", bufs=1) as const,
        tc.tile_pool(name="wpool", bufs=1) as wpool,
        tc.tile_pool(name="xrp", bufs=9) as xrp,
        tc.tile_pool(name="xhp", bufs=8) as xhp,
        tc.tile_pool(name="stats", bufs=8) as stats,
        tc.tile_pool(name="xtp", bufs=2) as xtp,
        tc.tile_pool(name="qtp", bufs=2) as qtp,
        tc.tile_pool(name="etp", bufs=4) as etp,
        tc.tile_pool(name="rrp", bufs=4) as rrp,
        tc.tile_pool(name="rbp", bufs=3) as rbp,
        tc.tile_pool(name="ctp", bufs=2) as ctp,
        tc.tile_pool(name="osp", bufs=2) as osp,
        tc.tile_pool(name="kvp", bufs=1) as kvp,
        tc.tile_pool(name="psum", bufs=8, space="PSUM") as psum,
    ):
        pools = {"stats": stats}

        # ---- constants / weights (one-time) ----
        ident = const.tile([128, 128], BF16, tag="ident")
        make_identity(nc, ident)
        eps_t = const.tile([128, 1], F32, tag="eps")
        nc.vector.memset(eps_t, EPS)
        pools["eps"] = eps_t
        ones_r = const.tile([1, 128], BF16, tag="ones")
        nc.vector.memset(ones_r, 1.0)
        ones_c = const.tile([128, 1], BF16, tag="onesc")
        nc.vector.memset(ones_c, 1.0)

        w_sb = {}
        for name, prm in (("wq", wq), ("wk", wk), ("wv", wv), ("wo", wo)):
            t = wpool.tile([128, KC, D], BF16, tag=name)
            nc.scalar.dma_start(out=t, in_=prm[:])
            w_sb[name] = t
        bq_sb = const.tile([128, KC], F32, tag="bq")
        nc.gpsimd.dma_start(out=bq_sb, in_=bq[:])
        bo_sb = const.tile([1, D], BF16, tag="bo")
        nc.gpsimd.dma_start(out=bo_sb, in_=bo[:])

        srow_sb = const.tile([1, NB], I32, tag="srow")
        nc.gpsimd.dma_start(out=srow_sb, in_=srow[:])
        rows = [
            nc.values_load(srow_sb[0:1, b:b + 1], min_val=0,
                           max_val=(PADF - WIN) * L,
                           skip_runtime_bounds_check=True)
            for b in range(NB)
        ]

        for b in range(NB):
            # ================= KV path (50 tokens) =================
            kv_raw = kvp.tile([KV, D], F32, tag="kv_raw")
            nc.gpsimd.dma_start(out=kv_raw, in_=aud[b, ds(rows[b], KV), :])

            rstd, nmr = _layernorm_stats(nc, pools, kv_raw, KV)
            kvh = kvp.tile([KV, D], BF16, tag="kvh")
            nc.scalar.activation(out=kvh, in_=kv_raw, func=AF.Identity,
                                 bias=nmr[:KV], scale=rstd[:KV])

            # kvT[kc] : [128, KV]  (feature-major normalized kv)
            kvT = kvp.tile([128, KC, KV], BF16, tag="kvT")
            for kc in range(KC):
                pt = psum.tile([128, 512], BF16, tag="ps")
                nc.tensor.transpose(pt[:, :KV], kvh[:, kc * 128:(kc + 1) * 128],
                                    ident[:KV, :KV])
                nc.vector.tensor_copy(out=kvT[:, kc, :], in_=pt[:, :KV])

            # k, v token-major [KV, D] (k bias dropped: softmax-invariant)
            k_tok = kvp.tile([KV, D], BF16, tag="k_tok")
            v_sb = kvp.tile([128, D], BF16, tag="v")
            for dst, wname in ((k_tok, "wk"), (v_sb, "wv")):
                for vc in range(D // TCH):
                    pv = psum.tile([128, 512], F32, tag="ps")
                    for kc in range(KC):
                        nc.tensor.matmul(pv[:KV, :], kvT[:, kc, :],
                                         w_sb[wname][:, kc, vc * TCH:(vc + 1) * TCH],
                                         start=(kc == 0), stop=(kc == KC - 1))
                    nc.vector.tensor_copy(out=dst[:KV, vc * TCH:(vc + 1) * TCH],
                                          in_=pv[:KV, :])
            # duplicate v at partitions 64.. so paired heads can contract there
            nc.sync.dma_start(out=v_sb[64:64 + KV, :], in_=v_sb[:KV, :])

            # kT[h] : [128, 64] per head (feature-major keys, padded 50->64 so
            # head pairs pack one PSUM bank with no junk rows)
            kT = kvp.tile([128, H, 64], BF16, tag="kT")
            nc.gpsimd.memset(kT, 0.0)
            for h in range(H):
                pt = psum.tile([128, 512], BF16, tag="ps")
                nc.tensor.transpose(pt[:, :KV], k_tok[:, h * 128:(h + 1) * 128],
                                    ident[:KV, :KV])
                nc.scalar.activation(out=kT[:, h, :KV], in_=pt[:, :KV],
                                     func=AF.Identity, bias=0.0, scale=1.0)

            # ====== LN for the whole batch up front (overlaps kv stage) ======
            xh_tiles = []
            xr_tiles = []
            for tt8 in range(LQ // 128):
                t0 = tt8 * 128
                # separate residual copy so the LN tile frees early
                x_r = xrp.tile([128, D], F32, tag="xr")
                xr_tiles.append(x_r)
                nc.sync.dma_start(out=x_r, in_=hid[b, t0:t0 + 128, :])

                rstd, nmr = _layernorm_stats(nc, pools, x_r, 128)
                xh = xhp.tile([128, D], BF16, tag="xh")
                nc.scalar.activation(out=xh, in_=x_r, func=AF.Identity,
                                     bias=nmr, scale=rstd)
                xh_tiles.append(xh)

            # ================= queries, 512-token chunks =================
            for ch in range(NCH):
                xt_c = xtp.tile([128, KC, TCH], BF16, tag="xt")
                x_tiles = xr_tiles[ch * TPC:(ch + 1) * TPC]
                for tt in range(TPC):
                    xh = xh_tiles[ch * TPC + tt]
                    # transpose 8 x [128,128] on PE, packed 4 per PSUM bank
                    for g in range(2):
                        pg = psum.tile([128, 512], BF16, tag="ps")
                        for j in range(4):
                            kc = g * 4 + j
                            nc.tensor.transpose(
                                pg[:, j * 128:(j + 1) * 128],
                                xh[:, kc * 128:(kc + 1) * 128], ident)
                        nc.scalar.activation(
                            out=xt_c[:, g * 4:(g + 1) * 4,
                                     tt * 128:(tt + 1) * 128],
                            in_=pg.rearrange("p (j f) -> p j f", j=4),
                            func=AF.Copy, bias=0.0, scale=1.0)

                # Q projection: qT_c[oc] = [128, TCH]
                qT_c = qtp.tile([128, KC, TCH], BF16, tag="qt")
                for oc in range(KC):
                    pq = psum.tile([128, TCH], F32, tag="ps")
                    for kc in range(KC):
                        nc.tensor.matmul(pq,
                                         w_sb["wq"][:, kc, oc * 128:(oc + 1) * 128],
                                         xt_c[:, kc, :],
                                         start=(kc == 0), stop=(kc == KC - 1))
                    nc.scalar.activation(out=qT_c[:, oc, :], in_=pq,
                                         func=AF.Identity,
                                         bias=bq_sb[:, oc:oc + 1], scale=1.0)

                # attention, head pairs: scores^T -> exp -> ctx^T (normalized).
                # Heads 2p/2p+1 land at PSUM partitions 0-63 / 64-127 (kT key
                # dim zero-padded to 64), so exp covers both in one full-lane
                # activation.
                ctxT = ctp.tile([128, KC, TCH], BF16, tag="ct")
                for p in range(H // 2):
                    h0, h1 = 2 * p, 2 * p + 1
                    sc = psum.tile([128, TCH], F32, tag="ps")
                    nc.tensor.matmul(sc[0:64, :], kT[:, h0, :], qT_c[:, h0, :],
                                     start=True, stop=True)
                    nc.tensor.matmul(sc[64:128, :], kT[:, h1, :],
                                     qT_c[:, h1, :], start=True, stop=True,
                                     tile_position=(0, 64))
                    expP = etp.tile([128, TCH], BF16, tag="et")
                    nc.scalar.activation(out=expP, in_=sc, func=AF.Exp,
                                         bias=0.0, scale=1.0)
                    for j, h in ((0, h0), (1, h1)):
                        sm = psum.tile([1, TCH], F32, tag="ps")
                        nc.tensor.matmul(
                            sm, ones_c[64 * j:64 * j + KV, :],
                            expP[64 * j:64 * j + KV, :], start=True, stop=True,
                            tile_position=(64 * j, 0))
                        rrow = rrp.tile([1, TCH], F32, tag="rr")
                        nc.vector.reciprocal_approx_fast(out=rrow, in_=sm)
                        rb = rbp.tile([128, TCH], F32, tag="rb")
                        nc.gpsimd.partition_broadcast(rb, rrow)
                        pc = psum.tile([128, TCH], F32, tag="ps")
                        nc.tensor.matmul(pc,
                                         v_sb[64 * j:64 * j + KV,
                                              h * 128:(h + 1) * 128],
                                         expP[64 * j:64 * j + KV, :],
                                         start=True, stop=True)
                        nc.vector.tensor_mul(out=ctxT[:, h, :], in0=pc, in1=rb)

                # out projection + bias + residual, per 128-token tile
                for tt in range(TPC):
                    t0 = ch * TCH + tt * 128
                    osb = osp.tile([128, D], F32, tag="os")
                    for vc in range(D // TCH):
                        po = psum.tile([128, TCH], F32, tag="ps")
                        for kc in range(KC):
                            nc.tensor.matmul(
                                po, ctxT[:, kc, tt * 128:(tt + 1) * 128],
                                w_sb["wo"][:, kc, vc * TCH:(vc + 1) * TCH],
                                start=(kc == 0), stop=False)
                        nc.tensor.matmul(po, ones_r,
                                         bo_sb[:, vc * TCH:(vc + 1) * TCH],
                                         start=False, stop=True)
                        nc.vector.tensor_add(
                            out=osb[:, vc * TCH:(vc + 1) * TCH], in0=po,
                            in1=x_tiles[tt][:, vc * TCH:(vc + 1) * TCH])
                    nc.sync.dma_start(out=out[b, t0:t0 + 128, :], in_=osb)


# ---------------------------------------------------------------------------
# host side
# ---------------------------------------------------------------------------

_PROG = None


def get_program() -> bass.Bass:
    global _PROG
    if _PROG is None:
        _PROG = build_program()
    return _PROG


def prep_inputs(hidden, audio_features, frame_idx, q_gamma, q_beta, kv_gamma,
                kv_beta, in_proj_w, in_proj_b, out_w, out_b):
    """Host-side sharding + parameter folding. Returns list of per-core maps."""
    hidden = np.asarray(hidden, np.float32)
    audio = np.asarray(audio_features, np.float32)
    fidx = np.asarray(frame_idx).astype(np.int64)
    q_gamma = np.asarray(q_gamma, np.float64)
    q_beta = np.asarray(q_beta, np.float64)
    kv_gamma = np.asarray(kv_gamma, np.float64)
    kv_beta = np.asarray(kv_beta, np.float64)
    w_in = np.asarray(in_proj_w, np.float64)
    b_in = np.asarray(in_proj_b, np.float64)
    w_out = np.asarray(out_w, np.float64)
    b_out = np.asarray(out_b, np.float64)

    Wq, Wk, Wv = w_in[:D], w_in[D:2 * D], w_in[2 * D:]
    bqv, bvv = b_in[:D], b_in[2 * D:]
    s = 1.0 / np.sqrt(HD)

    Wq_f = Wq * q_gamma[None, :] * s
    bq_f = (bqv + Wq @ q_beta) * s
    Wk_f = Wk * kv_gamma[None, :]
    Wv_f = Wv * kv_gamma[None, :]
    bv_f = bvv + Wv @ kv_beta
    bo_f = b_out + w_out @ bv_f

    def chunkT(w):  # [o,d] -> wT [d,o] -> [128, KC, D] (p, c, o)
        wt = np.ascontiguousarray(w.T).astype(np.float32)
        return np.ascontiguousarray(
            wt.reshape(KC, 128, D).transpose(1, 0, 2)).astype(ml_dtypes.bfloat16)

    wq_ship = chunkT(Wq_f)
    wk_ship = chunkT(Wk_f)
    wv_ship = chunkT(Wv_f)
    wo_ship = chunkT(w_out)

    bq_ship = np.ascontiguousarray(
        bq_f.astype(np.float32).reshape(KC, 128).T)
    bo_ship = bo_f.astype(np.float32).astype(ml_dtypes.bfloat16).reshape(1, D)

    # zero-padded audio: frame f -> rows (f+PAD_LO)*L ...; window start row =
    # (idx + WIN_LO + PAD_LO) * L = idx * L
    audio_pad = np.zeros((B, PADR, D), np.float32)
    audio_pad[:, PAD_LO * L:(PAD_LO + T) * L, :] = audio.reshape(B, T * L, D)
    start_row = (fidx * L).astype(np.int32)

    in_maps = []
    for c in range(NCORES):
        b0, b1 = c * NB, (c + 1) * NB
        in_maps.append({
            "hidden": hidden[b0:b1],
            "audio_pad": audio_pad[b0:b1],
            "start_row": start_row[b0:b1].reshape(1, NB),
            "wq": wq_ship, "wk": wk_ship, "wv": wv_ship, "wo": wo_ship,
            "bq": bq_ship, "bo": bo_ship,
        })
    return in_maps


def run(in_maps, **kwargs):
    nc = get_program()
    return run_bass_kernel_spmd(nc, in_maps, list(range(NCORES)), **kwargs)


def kernel(**inputs) -> np.ndarray:
    in_maps = prep_inputs(**inputs)
    res = run(in_maps)
    outs = [res.results[c]["out"] for c in range(NCORES)]
    return np.concatenate(outs, axis=0).astype(np.float32)


# revision 32
# speedup vs baseline: 1.0898x; 1.0599x over previous
# Trainium2 Bass kernel for AudioContextAggregation (windowed cross-attention).
#
# Strategy: data-parallel over batch B=16 across 8 NeuronCores (2 batches/core).
# Host folds LayerNorm gamma/beta and the 1/sqrt(hd) attention scale into the
# projection weights/biases (shipped as bf16), zero-pads audio along the frame
# axis so the per-batch window gather becomes a single dynamically-offset DMA,
# and pre-transposes all weights to the K-major layout the PE array wants.
# The K-projection bias is dropped exactly: it shifts every key score of a
# query by the same constant, which softmax cancels.
#
# Device pipeline per batch:
#   kv gather -> LN -> kvT -> K/V projections token-major (N=512) -> kT
#   per 512-token chunk of the 1024 queries:
#     LN(hidden) -> xhatT (DMA transpose) -> Q proj (bf16, N=512) ->
#     scores^T = kT.T @ qT (N=512) -> exp -> row-sums via ones-matmul ->
#     reciprocal -> broadcast-DMA -> ctx^T = (v.T @ expT) * recip ->
#     out proj (+bias via K=1 ones matmul) -> +hidden residual in fp32 -> out
# All matmuls run bf16 (1 cycle/row); LN stats, softmax sums and the residual
# stay fp32.

import numpy as np
import ml_dtypes

import concourse.bass as bass
import concourse.mybir as mybir
import concourse.tile as tile
from concourse import bacc
from concourse.bass import ds
from concourse.bass_utils import run_bass_kernel_spmd
from concourse.masks import make_identity

F32 = mybir.dt.float32
BF16 = mybir.dt.bfloat16
I32 = mybir.dt.int32
AF = mybir.ActivationFunctionType
ALU = mybir.AluOpType
AX = mybir.AxisListType

NCORES = 8
B, T, L, D = 16, 200, 5, 1024
LQ = 1024
H, HD = 8, 128
NB = B // NCORES            # batches per core
WIN_LO, WIN_HI = -4, 6
WIN = WIN_HI - WIN_LO       # 10 frames
KV = WIN * L                # 50 kv tokens
PAD_LO = -WIN_LO            # 4 zero frames in front
PADF = PAD_LO + T + WIN_HI  # 210 padded frames
PADR = PADF * L             # 1050 padded rows
KC = D // 128               # 8 feature chunks of 128
TCH = 512                   # token chunk (matmul N)
NCH = LQ // TCH             # chunks per batch
TPC = TCH // 128            # 128-token tiles per chunk
EPS = 1e-5


def _bcast_part(ap: bass.AP, p: int) -> bass.AP:
    """View a [1, N] AP as [p, N] with partition stride 0 (DMA broadcast)."""
    return bass.AP(tensor=ap.tensor, offset=ap.offset, ap=[[0, p]] + ap.ap[1:])


def _layernorm_stats(nc, pools, x_ap, p):
    """Emit LN stats for x_ap [p, D] (f32). Returns (rstd, neg_mu_rstd) [p,1]."""
    st = pools["stats"]
    nsub = D // 512
    stats = st.tile([128, nsub, 6], F32, tag="bnst")
    xg = x_ap.rearrange("p (s d) -> p s d", s=nsub)
    for s in range(nsub):
        nc.vector.bn_stats(out=stats[:p, s, :], in_=xg[:, s, :])
    mv = st.tile([128, 2], F32, tag="bnmv")
    nc.vector.bn_aggr(out=mv[:p], in_=stats[:p])
    rstd = st.tile([128, 1], F32, tag="rstd")
    # rstd = 1/sqrt(var + eps)
    nc.scalar.activation(out=rstd[:p], in_=mv[:p, 1:2], func=AF.Sqrt,
                         bias=pools["eps"][:p], scale=1.0)
    nc.vector.reciprocal(out=rstd[:p], in_=rstd[:p])
    nmr = st.tile([128, 1], F32, tag="nmr")
    # nmr = -(mean * rstd)
    nc.vector.tensor_scalar(out=nmr[:p], in0=mv[:p, 0:1], scalar1=rstd[:p],
                            scalar2=-1.0, op0=ALU.mult, op1=ALU.mult)
    return rstd, nmr


def build_program() -> bass.Bass:
    nc = bacc.Bacc("TRN2", target_bir_lowering=False, debug=False)

    hid = nc.declare_dram_parameter("hidden", [NB, LQ, D], F32, isOutput=False)
    aud = nc.declare_dram_parameter("audio_pad", [NB, PADR, D], F32, isOutput=False)
    srow = nc.declare_dram_parameter("start_row", [1, NB], I32, isOutput=False)
    wq = nc.declare_dram_parameter("wq", [128, KC, D], BF16, isOutput=False)
    wk = nc.declare_dram_parameter("wk", [128, KC, D], BF16, isOutput=False)
    wv = nc.declare_dram_parameter("wv", [128, KC, D], BF16, isOutput=False)
    wo = nc.declare_dram_parameter("wo", [128, KC, D], BF16, isOutput=False)
    bq = nc.declare_dram_parameter("bq", [128, KC], F32, isOutput=False)
    bo = nc.declare_dram_parameter("bo", [1, D], BF16, isOutput=False)
    out = nc.declare_dram_parameter("out", [NB, LQ, D], F32, isOutput=True)

    with tile.TileContext(nc) as tc:
        _emit(tc, hid, aud, srow, wq, wk, wv, wo, bq, bo, out)
    nc.compile()
    return nc


def _emit(tc, hid, aud, srow, wq, wk, wv, wo, bq, bo, out):
    nc = tc.nc

    with (
        tc.tile_pool(name="const", bufs=1) as const,
        tc.tile_pool(name="wpool", bufs=1) as wpool,
        tc.tile_pool(name="xrp", bufs=9) as xrp,
        tc.tile_pool(name="xhp", bufs=8) as xhp,
        tc.tile_pool(name="stats", bufs=8) as stats,
        tc.tile_pool(name="xtp", bufs=2) as xtp,
        tc.tile_pool(name="qtp", bufs=2) as qtp,
        tc.tile_pool(name="etp", bufs=4) as etp,
        tc.tile_pool(name="rrp", bufs=4) as rrp,
        tc.tile_pool(name="rbp", bufs=3) as rbp,
        tc.tile_pool(name="ctp", bufs=2) as ctp,
        tc.tile_pool(name="osp", bufs=2) as osp,
        tc.tile_pool(name="kvp", bufs=1) as kvp,
        tc.tile_pool(name="psum", bufs=8, space="PSUM") as psum,
    ):
        pools = {"stats": stats}

        # ---- constants / weights (one-time) ----
        ident = const.tile([128, 128], BF16, tag="ident")
        make_identity(nc, ident)
        eps_t = const.tile([128, 1], F32, tag="eps")
        nc.vector.memset(eps_t, EPS)
        pools["eps"] = eps_t
        ones_r = const.tile([1, 128], BF16, tag="ones")
        nc.vector.memset(ones_r, 1.0)
        ones_c = const.tile([128, 1], BF16, tag="onesc")
        nc.vector.memset(ones_c, 1.0)

        w_sb = {}
        for name, prm in (("wq", wq), ("wk", wk), ("wv", wv), ("wo", wo)):
            t = wpool.tile([128, KC, D], BF16, tag=name)
            nc.scalar.dma_start(out=t, in_=prm[:])
            w_sb[name] = t
        bq_sb = const.tile([128, KC], F32, tag="bq")
        nc.gpsimd.dma_start(out=bq_sb, in_=bq[:])
        bo_sb = const.tile([1, D], BF16, tag="bo")
        nc.gpsimd.dma_start(out=bo_sb, in_=bo[:])

        srow_sb = const.tile([1, NB], I32, tag="srow")
        nc.gpsimd.dma_start(out=srow_sb, in_=srow[:])
        rows = [
            nc.values_load(srow_sb[0:1, b:b + 1], min_val=0,
                           max_val=(PADF - WIN) * L,
                           skip_runtime_bounds_check=True)
            for b in range(NB)
        ]

        for b in range(NB):
            # ================= KV path (50 tokens) =================
            kv_raw = kvp.tile([KV, D], F32, tag="kv_raw")
            nc.gpsimd.dma_start(out=kv_raw, in_=aud[b, ds(rows[b], KV), :])

            rstd, nmr = _layernorm_stats(nc, pools, kv_raw, KV)
            kvh = kvp.tile([KV, D], BF16, tag="kvh")
            nc.scalar.activation(out=kvh, in_=kv_raw, func=AF.Identity,
                                 bias=nmr[:KV], scale=rstd[:KV])

            # kvT[kc] : [128, KV]  (feature-major normalized kv)
            kvT = kvp.tile([128, KC, KV], BF16, tag="kvT")
            for kc in range(KC):
                pt = psum.tile([128, 512], BF16, tag="ps")
                nc.tensor.transpose(pt[:, :KV], kvh[:, kc * 128:(kc + 1) * 128],
                                    ident[:KV, :KV])
                nc.vector.tensor_copy(out=kvT[:, kc, :], in_=pt[:, :KV])

            # k, v token-major [KV, D] (k bias dropped: softmax-invariant)
            k_tok = kvp.tile([KV, D], BF16, tag="k_tok")
            v_sb = kvp.tile([128, D], BF16, tag="v")
            for dst, wname in ((k_tok, "wk"), (v_sb, "wv")):
                for vc in range(D // TCH):
                    pv = psum.tile([128, 512], F32, tag="ps")
                    for kc in range(KC):
                        nc.tensor.matmul(pv[:KV, :], kvT[:, kc, :],
                                         w_sb[wname][:, kc, vc * TCH:(vc + 1) * TCH],
                                         start=(kc == 0), stop=(kc == KC - 1))
                    nc.vector.tensor_copy(out=dst[:KV, vc * TCH:(vc + 1) * TCH],
                                          in_=pv[:KV, :])
            # duplicate v at partitions 64.. so paired heads can contract there
            nc.sync.dma_start(out=v_sb[64:64 + KV, :], in_=v_sb[:KV, :])

            # kT[h] : [128, 64] per head (feature-major keys, padded 50->64 so
            # head pairs pack one PSUM bank with no junk rows)
            kT = kvp.tile([128, H, 64], BF16, tag="kT")
            nc.gpsimd.memset(kT, 0.0)
            for h in range(H):
                pt = psum.tile([128, 512], BF16, tag="ps")
                nc.tensor.transpose(pt[:, :KV], k_tok[:, h * 128:(h + 1) * 128],
                                    ident[:KV, :KV])
                nc.scalar.activation(out=kT[:, h, :KV], in_=pt[:, :KV],
                                     func=AF.Identity, bias=0.0, scale=1.0)

            # ====== LN for the whole batch up front (overlaps kv stage) ======
            xh_tiles = []
            xr_tiles = []
            for tt8 in range(LQ // 128):
                t0 = tt8 * 128
                # separate residual copy so the LN tile frees early
                x_r = xrp.tile([128, D], F32, tag="xr")
                xr_tiles.append(x_r)
                nc.sync.dma_start(out=x_r, in_=hid[b, t0:t0 + 128, :])

                rstd, nmr = _layernorm_stats(nc, pools, x_r, 128)
                xh = xhp.tile([128, D], BF16, tag="xh")
                nc.scalar.activation(out=xh, in_=x_r, func=AF.Identity,
                                     bias=nmr, scale=rstd)
                xh_tiles.append(xh)

            # ================= queries, 512-token chunks =================
            for ch in range(NCH):
                xt_c = xtp.tile([128, KC, TCH], BF16, tag="xt")
                x_tiles = xr_tiles[ch * TPC:(ch + 1) * TPC]
                for tt in range(TPC):
                    xh = xh_tiles[ch * TPC + tt]
                    # transpose 8 x [128,128] on PE, packed 4 per PSUM bank
                    for g in range(2):
                        pg = psum.tile([128, 512], BF16, tag="ps")
                        for j in range(4):
                            kc = g * 4 + j
                            nc.tensor.transpose(
                                pg[:, j * 128:(j + 1) * 128],
                                xh[:, kc * 128:(kc + 1) * 128], ident)
                        nc.scalar.activation(
                            out=xt_c[:, g * 4:(g + 1) * 4,
                                     tt * 128:(tt + 1) * 128],
                            in_=pg.rearrange("p (j f) -> p j f", j=4),
                            func=AF.Copy, bias=0.0, scale=1.0)

                # Q projection: qT_c[oc] = [128, TCH]
                qT_c = qtp.tile([128, KC, TCH], BF16, tag="qt")
                for oc in range(KC):
                    pq = psum.tile([128, TCH], F32, tag="ps")
                    for kc in range(KC):
                        nc.tensor.matmul(pq,
                                         w_sb["wq"][:, kc, oc * 128:(oc + 1) * 128],
                                         xt_c[:, kc, :],
                                         start=(kc == 0), stop=(kc == KC - 1))
                    nc.scalar.activation(out=qT_c[:, oc, :], in_=pq,
                                         func=AF.Identity,
                                         bias=bq_sb[:, oc:oc + 1], scale=1.0)

                # attention, head pairs: scores^T -> exp -> ctx^T (normalized).
                # Heads 2p/2p+1 land at PSUM partitions 0-63 / 64-127 (kT key
                # dim zero-padded to 64), so exp covers both in one full-lane
                # activation.
                ctxT = ctp.tile([128, KC, TCH], BF16, tag="ct")
                for p in range(H // 2):
                    h0, h1 = 2 * p, 2 * p + 1
                    sc = psum.tile([128, TCH], F32, tag="ps")
                    nc.tensor.matmul(sc[0:64, :], kT[:, h0, :], qT_c[:, h0, :],
                                     start=True, stop=True)
                    nc.tensor.matmul(sc[64:128, :], kT[:, h1, :],
                                     qT_c[:, h1, :], start=True, stop=True,
                                     tile_position=(0, 64))
                    expP = etp.tile([128, TCH], BF16, tag="et")
                    nc.scalar.activation(out=expP, in_=sc, func=AF.Exp,
                                         bias=0.0, scale=1.0)
                    for j, h in ((0, h0), (1, h1)):
                        # sums overwrite a row of the (already-consumed)
                        # scores bank instead of burning a fresh PSUM slot
                        sm = sc[64 * j:64 * j + 1, :]
                        nc.tensor.matmul(
                            sm, ones_c[64 * j:64 * j + KV, :],
                            expP[64 * j:64 * j + KV, :], start=True, stop=True,
                            tile_position=(64 * j, 64 * j))
                        rrow = rrp.tile([1, TCH], F32, tag="rr")
                        nc.vector.reciprocal_approx_fast(out=rrow, in_=sm)
                        rb = rbp.tile([128, TCH], F32, tag="rb")
                        nc.gpsimd.partition_broadcast(rb, rrow)
                        pc = psum.tile([128, TCH], F32, tag="ps")
                        nc.tensor.matmul(pc,
                                         v_sb[64 * j:64 * j + KV,
                                              h * 128:(h + 1) * 128],
                                         expP[64 * j:64 * j + KV, :],
                                         start=True, stop=True)
                        nc.vector.tensor_mul(out=ctxT[:, h, :], in0=pc, in1=rb)

                # out projection + bias + residual, per 128-token tile
                for tt in range(TPC):
                    t0 = ch * TCH + tt * 128
                    osb = osp.tile([128, D], F32, tag="os")
                    for vc in range(D // TCH):
                        po = psum.tile([128, TCH], F32, tag="ps")
                        for kc in range(KC):
                            nc.tensor.matmul(
                                po, ctxT[:, kc, tt * 128:(tt + 1) * 128],
                                w_sb["wo"][:, kc, vc * TCH:(vc + 1) * TCH],
                                start=(kc == 0), stop=False)
                        nc.tensor.matmul(po, ones_r,
                                         bo_sb[:, vc * TCH:(vc + 1) * TCH],
                                         start=False, stop=True)
                        nc.vector.tensor_add(
                            out=osb[:, vc * TCH:(vc + 1) * TCH], in0=po,
                            in1=x_tiles[tt][:, vc * TCH:(vc + 1) * TCH])
                    nc.sync.dma_start(out=out[b, t0:t0 + 128, :], in_=osb)


# ---------------------------------------------------------------------------
# host side
# ---------------------------------------------------------------------------

_PROG = None


def get_program() -> bass.Bass:
    global _PROG
    if _PROG is None:
        _PROG = build_program()
    return _PROG


def prep_inputs(hidden, audio_features, frame_idx, q_gamma, q_beta, kv_gamma,
                kv_beta, in_proj_w, in_proj_b, out_w, out_b):
    """Host-side sharding + parameter folding. Returns list of per-core maps."""
    hidden = np.asarray(hidden, np.float32)
    audio = np.asarray(audio_features, np.float32)
    fidx = np.asarray(frame_idx).astype(np.int64)
    q_gamma = np.asarray(q_gamma, np.float64)
    q_beta = np.asarray(q_beta, np.float64)
    kv_gamma = np.asarray(kv_gamma, np.float64)
    kv_beta = np.asarray(kv_beta, np.float64)
    w_in = np.asarray(in_proj_w, np.float64)
    b_in = np.asarray(in_proj_b, np.float64)
    w_out = np.asarray(out_w, np.float64)
    b_out = np.asarray(out_b, np.float64)

    Wq, Wk, Wv = w_in[:D], w_in[D:2 * D], w_in[2 * D:]
    bqv, bvv = b_in[:D], b_in[2 * D:]
    s = 1.0 / np.sqrt(HD)

    Wq_f = Wq * q_gamma[None, :] * s
    bq_f = (bqv + Wq @ q_beta) * s
    Wk_f = Wk * kv_gamma[None, :]
    Wv_f = Wv * kv_gamma[None, :]
    bv_f = bvv + Wv @ kv_beta
    bo_f = b_out + w_out @ bv_f

    def chunkT(w):  # [o,d] -> wT [d,o] -> [128, KC, D] (p, c, o)
        wt = np.ascontiguousarray(w.T).astype(np.float32)
        return np.ascontiguousarray(
            wt.reshape(KC, 128, D).transpose(1, 0, 2)).astype(ml_dtypes.bfloat16)

    wq_ship = chunkT(Wq_f)
    wk_ship = chunkT(Wk_f)
    wv_ship = chunkT(Wv_f)
    wo_ship = chunkT(w_out)

    bq_ship = np.ascontiguousarray(
        bq_f.astype(np.float32).reshape(KC, 128).T)
    bo_ship = bo_f.astype(np.float32).astype(ml_dtypes.bfloat16).reshape(1, D)

    # zero-padded audio: frame f -> rows (f+PAD_LO)*L ...; window start row =
    # (idx + WIN_LO + PAD_LO) * L = idx * L
    audio_pad = np.zeros((B, PADR, D), np.float32)
    audio_pad[:, PAD_LO * L:(PAD_LO + T) * L, :] = audio.reshape(B, T * L, D)
    start_row = (fidx * L).astype(np.int32)

    in_maps = []
    for c in range(NCORES):
        b0, b1 = c * NB, (c + 1) * NB
        in_maps.append({
            "hidden": hidden[b0:b1],
            "audio_pad": audio_pad[b0:b1],
            "start_row": start_row[b0:b1].reshape(1, NB),
            "wq": wq_ship, "wk": wk_ship, "wv": wv_ship, "wo": wo_ship,
            "bq": bq_ship, "bo": bo_ship,
        })
    return in_maps


def run(in_maps, **kwargs):
    nc = get_program()
    return run_bass_kernel_spmd(nc, in_maps, list(range(NCORES)), **kwargs)


def kernel(**inputs) -> np.ndarray:
    in_maps = prep_inputs(**inputs)
    res = run(in_maps)
    outs = [res.results[c]["out"] for c in range(NCORES)]
    return np.concatenate(outs, axis=0).astype(np.float32)
